# revision 1
# baseline (speedup 1.0000x reference)
"""Trainium2 SPMD kernel for nn_AutoregressiveDecoder (gnn_message_passing).

Math (reference, per context g in 0..N-1, N=384):
    h1[g]  = concat(z, e_g) @ W1 = H0 + e_g (x) W1r     # H0 = z @ W1[:128]
    A[g]   = relu(P_g @ h1[g])         P_g = partials[g]
    h2[g]  = A[g] @ W2
    h3[g]  = P_g @ h2[g]
    S[g,:] = h3[g][g,:] @ h3[g].T      (row g of supplement, pre-tril)
    out    = x + 0.5*(tril(S) + tril(S).T)

8 cores x 48 contexts, raw Bass (manual semaphores), fp32r matmuls except
the A@W2 stage in bf16.  Per context i (software-pipelined, skew 3):
    mm1  A_T[h,:]  = sum_j H0[j,h] Pt[j,:]  (+ rank-1 W1r (x) pcol)  N=384
    mm2  h2[j,k]   = sum_h A_T[h,j] W2[h,k]                          N=128 bf16
    mm3  h3T[k,:]  = sum_j h2[j,k] PtAug[j,:]  (col 384 = d vector)  N=385
    mm4  S[1,:]    = sum_k d[k] h3T[k,:]   (psum row aliased in h3ps) N=384
Pt = P_g.T pre-transposed on host; PtAug's col 384 is P_g[g,:] so mm3 also
yields d = h3[g][g,:].  tril/symmetrize/(+x) happen on host at unshard.
PE stream at iter i: mm1(i), mm2(i-1), mm3(i-2), mm4(i-3) -- the ACT/DVE
relu/copies of stage k run a full iteration before their PE consumer.
"""

import os
from contextlib import ExitStack

import numpy as np
import ml_dtypes

import concourse.bass as bass
import concourse.mybir as mybir
from concourse.bass_utils import run_bass_kernel_spmd

N = 384
D = 128
HID = 256
HID2 = 128
NCORES = 8
NB = N // NCORES  # 48 contexts per core
W = N + 2  # pt width: prow column at N, plus even-size pad (fp32r dst rule)
PTBUF = 8  # pt SBUF ring depth
SRBUF = 8  # S-row SBUF ring depth

F32 = mybir.dt.float32
F32R = mybir.dt.float32r
BF16 = mybir.dt.bfloat16
AFT = mybir.ActivationFunctionType

_NC_CACHE = {}
LAST_RESULT = None  # test.py reads exec_time_ns from here


def _round_f32r(a: np.ndarray) -> np.ndarray:
    """Round fp32 to fp32r (TF32-like: low 12 mantissa bits cleared, RNE)."""
    u = np.ascontiguousarray(a, dtype=np.float32).view(np.uint32)
    add = np.uint32(0x7FF) + ((u >> np.uint32(12)) & np.uint32(1))
    r = (u + add) & np.uint32(0xFFFFF000)
    return r.view(np.float32)


def _build_nc() -> bass.Bass:
    nc = bass.Bass()
    pt_d = nc.declare_dram_parameter("pt", [NB, 128, 3 * W], BF16, isOutput=False)
    pcol_d = nc.declare_dram_parameter("pcol", [1, NB * N], BF16, isOutput=False)
    h0f_d = nc.declare_dram_parameter("h0f", [128, 3 * HID], BF16, isOutput=False)
    w1r_d = nc.declare_dram_parameter("w1r", [1, HID], BF16, isOutput=False)
    w2f_d = nc.declare_dram_parameter("w2f", [128, 2 * HID2], BF16, isOutput=False)
    out_ds = [
        nc.declare_dram_parameter(f"o{b:02d}", [1, N], F32, isOutput=True)
        for b in range(NB)
    ]

    ctx = ExitStack()
    with ctx:
        # ---- persistent SBUF ----
        h0f = ctx.enter_context(nc.sbuf_tensor("h0f_s", [128, 3 * HID], BF16))
        w1r = ctx.enter_context(nc.sbuf_tensor("w1r_s", [1, HID], BF16))
        w2f = ctx.enter_context(nc.sbuf_tensor("w2f_s", [128, 2 * HID2], BF16))
        pcall = ctx.enter_context(nc.sbuf_tensor("pcall_s", [1, NB * N], BF16))
        pt = [
            ctx.enter_context(nc.sbuf_tensor(f"ptb{s}", [128, 3 * W], BF16))
            for s in range(PTBUF)
        ]
        at = [
            ctx.enter_context(nc.sbuf_tensor(f"atb{s}", [128, 2 * N], BF16))
            for s in range(3)
        ]
        h2sb = [
            ctx.enter_context(nc.sbuf_tensor(f"h2b{s}", [128, N], BF16))
            for s in range(3)
        ]
        h3sb = [
            ctx.enter_context(nc.sbuf_tensor(f"h3b{s}", [128, W], BF16))
            for s in range(3)
        ]
        srow = [
            ctx.enter_context(nc.sbuf_tensor(f"srowb{s}", [1, N], F32))
            for s in range(SRBUF)
        ]
        # ---- PSUM: 8 banks exactly ----
        aps = [
            [
                ctx.enter_context(
                    nc.psum_tensor(f"apsb{p}{h}", [128, N], F32)
                )
                for h in range(2)
            ]
            for p in range(2)
        ]  # aps[pair][hc]
        h2ps = [
            ctx.enter_context(nc.psum_tensor(f"h2psb{s}", [128, N], F32))
            for s in range(2)
        ]
        h3ps = [
            ctx.enter_context(nc.psum_tensor(f"h3psb{s}", [128, W], F32))
            for s in range(2)
        ]

        # ---- semaphores ----
        sem_const = ctx.enter_context(nc.semaphore("sem_const"))
        sem_pc2 = ctx.enter_context(nc.semaphore("sem_pc2"))
        sem_w2 = ctx.enter_context(nc.semaphore("sem_w2"))
        sem_pt = [
            ctx.enter_context(nc.semaphore(f"sem_pt{s}")) for s in range(PTBUF)
        ]
        sem_out = [
            ctx.enter_context(nc.semaphore(f"sem_out{s}")) for s in range(SRBUF)
        ]
        sem_mm1 = ctx.enter_context(nc.semaphore("sem_mm1"))
        sem_relu = ctx.enter_context(nc.semaphore("sem_relu"))
        sem_mm2 = ctx.enter_context(nc.semaphore("sem_mm2"))
        sem_h2c = ctx.enter_context(nc.semaphore("sem_h2c"))
        sem_mm3 = ctx.enter_context(nc.semaphore("sem_mm3"))
        sem_h3c = ctx.enter_context(nc.semaphore("sem_h3c"))
        sem_mm4 = ctx.enter_context(nc.semaphore("sem_mm4"))
        sem_sc = ctx.enter_context(nc.semaphore("sem_sc"))

        block = ctx.enter_context(nc.Block())

        NI = NB + 3  # pipeline iterations (skew 3)

        PCA = 8  # contexts whose pcol rows load before the loop starts

        @block.sync
        def _(sync):
            sync.dma_start(h0f[:, :], h0f_d[:, :]).then_inc(sem_const, 16)
            sync.dma_start(w1r[:, :], w1r_d[:, :]).then_inc(sem_const, 16)
            sync.dma_start(
                pcall[:, 0 : PCA * N], pcol_d[:, 0 : PCA * N]
            ).then_inc(sem_const, 16)
            sync.dma_start(w2f[:, 0:HID2], w2f_d[:, 0:HID2]).then_inc(sem_w2, 16)
            for i in range(NI):
                k = i - 3
                if 0 <= k < NB:
                    sync.wait_ge(sem_sc, k + 1)
                    sync.dma_start(
                        out_ds[k][:, :], srow[k % SRBUF][:, :]
                    ).then_inc(sem_out[k % SRBUF], 16)



        @block.gpsimd
        def _(g):
            for p in range(min(PTBUF, NB)):
                if p >= 2:
                    # keep only 2 prefetch DMAs in flight so pt(0) is not
                    # bandwidth-shared 6 ways (rings interleave packets)
                    g.wait_ge(sem_pt[p - 2], 16)
                g.dma_start(pt[p][:, :], pt_d[p]).then_inc(sem_pt[p], 16)
            for i in range(NI):
                p = i + PTBUF
                if p < NB:
                    g.wait_ge(sem_mm3, i + 1)
                    g.dma_start(
                        pt[p % PTBUF][:, :], pt_d[p]
                    ).then_inc(sem_pt[p % PTBUF], 16)


        @block.tensor
        def _(te):
            te.wait_ge(sem_const, 48)
            for i in range(NI):
                if i == 1:
                    te.wait_ge(sem_w2, 32)
                if i == PCA:
                    te.wait_ge(sem_pc2, 16)
                # ---- mm1(i): A_T chunks + rank-1, fp32r N=384 ----
                if i < NB:
                    # aps-pair-reuse wait (sem_relu >= 2i-2) is implied by the
                    # previous iteration's wait before mm2.
                    te.wait_ge(sem_pt[i % PTBUF], 16 * (i // PTBUF + 1))
                    ptt = pt[i % PTBUF]
                    for hc in range(2):
                        for t in range(3):
                            nc.tensor.matmul(
                                aps[i % 2][hc][:, :],
                                h0f[:, t * HID + hc * 128 : t * HID + hc * 128 + 128],
                                ptt[:, t * W : t * W + N],
                                start=(t == 0),
                                stop=False,
                                skip_group_check=True,
                            )

                # ---- mm2(i-1): h2 = A@W2, bf16 N=128 ----
                k = i - 1
                if 0 <= k < NB:
                    te.wait_ge(sem_relu, 2 * k + 2)
                    # h2ps[k%2]-reuse wait is implied by the previous
                    # iteration's wait before mm3.
                    dst = h2ps[k % 2]
                    for jc in range(3):
                        for ht in range(2):
                            mm = nc.tensor.matmul(
                                dst[:, jc * 128 : (jc + 1) * 128],
                                at[k % 3][
                                    :, ht * N + jc * 128 : ht * N + jc * 128 + 128
                                ],
                                w2f[:, ht * HID2 : (ht + 1) * HID2],
                                start=(ht == 0),
                                stop=(ht == 1),
                            )
                    if not (0 <= i - 2 < NB):
                        mm.then_inc(sem_mm2, 1)  # no mm3 rider this iter
                # ---- rank-1 pair for mm1(i), emitted after the short bf16
                # matmuls: the K=1 weight loads can't prefetch past a
                # full-height in-flight matmul, so placing them here turns two
                # ~250ns stalls into one small one ----
                if i < NB:
                    r1 = []
                    for hc in range(2):
                        r1.append(
                            nc.tensor.matmul(
                                aps[i % 2][hc][:, :],
                                w1r[:, hc * 128 : (hc + 1) * 128],
                                pcall[:, i * N : (i + 1) * N],
                                start=False,
                                stop=True,
                                skip_group_check=True,
                            )
                        )
                    if i < 2:
                        r1[0].then_inc(sem_mm1, 1)
                        r1[1].then_inc(sem_mm1, 1)
                    else:
                        r1[1].then_inc(sem_mm1, 1)  # hc0 group drained
                        # hc1's inc rides on mm3(i-2)-t1 below
                # ---- mm3(i-2): h3T (+d col), N=386 ----
                k = i - 2
                if 0 <= k < NB:
                    te.wait_ge(sem_h2c, k + 1)
                    if k >= 2:
                        # h3ps[k%2]-reuse is implied by last iter's mm4 wait
                        te.wait_ge(sem_sc, k - 1)  # aliased S row was drained
                    dst = h3ps[k % 2]
                    ptt = pt[k % PTBUF]
                    for t in range(3):
                        mm = nc.tensor.matmul(
                            dst[:, :],
                            h2sb[k % 3][:, t * 128 : (t + 1) * 128],
                            ptt[:, t * W : (t + 1) * W],
                            start=(t == 0),
                            stop=(t == 2),
                        )
                        if t == 0 and k + 1 < NB:
                            # completion implies same-iter mm2(k+1) drained
                            mm.then_inc(sem_mm2, 1)
                        if t == 1 and i < NB:
                            # completion implies same-iter rank-1 hc1 drained
                            mm.then_inc(sem_mm1, 1)
                    mm.then_inc(sem_mm3, 1)
                # ---- mm4(i-3): S row into h3ps[k%2] partition 0 ----
                k = i - 3
                if 0 <= k < NB:
                    te.wait_ge(sem_h3c, k + 1)
                    mm = nc.tensor.matmul(
                        h3ps[k % 2][0:1, 0:N],
                        h3sb[k % 3][:, N : N + 1],
                        h3sb[k % 3][:, 0:N],
                        start=True,
                        stop=True,
                    )
                    mm.then_inc(sem_mm4, 1)

        @block.scalar
        def _(sc):
            sc.dma_start(w2f[:, HID2:], w2f_d[:, HID2:]).then_inc(sem_w2, 16)
            sc.dma_start(
                pcall[:, PCA * N :], pcol_d[:, PCA * N :]
            ).then_inc(sem_pc2, 16)
            for i in range(NI):
                k = i
                if k < NB:
                    if k >= 3:
                        sc.wait_ge(sem_mm2, k - 2)  # at[k%3] reuse
                    for hc in range(2):
                        sc.wait_ge(sem_mm1, 2 * k + hc + 1)
                        nc.scalar.activation(
                            at[k % 3][:, hc * N : (hc + 1) * N],
                            aps[k % 2][hc][:, :],
                            AFT.Relu,
                        ).then_inc(sem_relu, 1)


        @block.vector
        def _(ve):
            for i in range(NI):
                k = i - 1
                if 0 <= k < NB:
                    if k >= 3:
                        ve.wait_ge(sem_mm3, k - 2)  # h2sb[k%3] reuse
                    ve.wait_ge(sem_mm2, k + 1)
                    nc.vector.tensor_copy(
                        h2sb[k % 3][:, :], h2ps[k % 2][:, :]
                    ).then_inc(sem_h2c, 1)
                k = i - 2
                if 0 <= k < NB:
                    if k >= 3:
                        ve.wait_ge(sem_mm4, k - 2)  # h3sb[k%3] reuse
                    ve.wait_ge(sem_mm3, k + 1)
                    nc.vector.tensor_copy(
                        h3sb[k % 3][:, :], h3ps[k % 2][:, :]
                    ).then_inc(sem_h3c, 1)
                k = i - 3
                if 0 <= k < NB:
                    ve.wait_ge(sem_mm4, k + 1)
                    if k >= SRBUF:
                        ve.wait_ge(sem_out[k % SRBUF], 16 * (k // SRBUF))
                    nc.vector.tensor_copy(
                        srow[k % SRBUF][:, :], h3ps[k % 2][0:1, 0:N]
                    ).then_inc(sem_sc, 1)

    return nc


def _get_nc() -> bass.Bass:
    if "nc" not in _NC_CACHE:
        _NC_CACHE["nc"] = _build_nc()
    return _NC_CACHE["nc"]


def kernel(z, x, partials, W1, W2):
    global LAST_RESULT
    z = np.asarray(z, dtype=np.float32)
    x = np.asarray(x, dtype=np.float32)
    partials = np.asarray(partials, dtype=np.float32)
    W1 = np.asarray(W1, dtype=np.float32)
    W2 = np.asarray(W2, dtype=np.float32)

    H0 = z[0] @ W1[:D]  # [384, 256]
    h0f = (
        np.ascontiguousarray(H0.reshape(3, 128, HID).transpose(1, 0, 2))
        .reshape(128, 3 * HID)
        .astype(ml_dtypes.bfloat16)
    )
    w1r = np.ascontiguousarray(W1[D : D + 1]).astype(ml_dtypes.bfloat16)
    w2f = (
        np.ascontiguousarray(W2.reshape(2, 128, HID2).transpose(1, 0, 2))
        .reshape(128, 2 * HID2)
        .astype(ml_dtypes.bfloat16)
    )

    ptT = np.ascontiguousarray(partials.transpose(0, 2, 1))  # ptT[g,j,i]=P_g[i,j]
    ar = np.arange(N)
    prow = partials[ar, ar, :]  # [384, 384]  P_g[g, :]
    pcol = ptT[ar, ar, :]  # [384, 384]  P_g[:, g]

    in_maps = []
    for c in range(NCORES):
        gs = slice(c * NB, (c + 1) * NB)
        aug = np.zeros((NB, 3, 128, W), dtype=ml_dtypes.bfloat16)
        aug[..., :N] = ptT[gs].reshape(NB, 3, 128, N).astype(ml_dtypes.bfloat16)
        aug[..., N] = prow[gs].reshape(NB, 3, 128).astype(ml_dtypes.bfloat16)
        aug = np.ascontiguousarray(aug.transpose(0, 2, 1, 3)).reshape(NB, 128, 3 * W)
        in_maps.append(
            {
                "pt": aug,
                "pcol": np.ascontiguousarray(pcol[gs])
                .astype(ml_dtypes.bfloat16)
                .reshape(1, NB * N),
                "h0f": h0f,
                "w1r": w1r,
                "w2f": w2f,
            }
        )

    nc = _get_nc()
    res = run_bass_kernel_spmd(
        nc,
        in_maps,
        core_ids=list(range(NCORES)),
        trace=bool(os.environ.get("KERNEL_TRACE")),
    )
    LAST_RESULT = res
    S = np.concatenate(
        [
            np.concatenate(
                [
                    np.asarray(res.results[c][f"o{b:02d}"], np.float32)
                    for b in range(NB)
                ],
                axis=0,
            )
            for c in range(NCORES)
        ],
        axis=0,
    )  # [384, 384] raw supplement rows
    sup = np.tril(S)
    sup = (sup + sup.T) * np.float32(0.5)
    return (x + sup).astype(np.float32)



# revision 6
# speedup vs baseline: 1.0861x; 1.0861x over previous
"""Trainium2 SPMD kernel for nn_AutoregressiveDecoder (gnn_message_passing).

Math (reference, per context g in 0..N-1, N=384):
    h1[g]  = concat(z, e_g) @ W1 = H0 + e_g (x) W1r     # H0 = z @ W1[:128]
    A[g]   = relu(P_g @ h1[g])         P_g = partials[g]
    h2[g]  = A[g] @ W2
    h3[g]  = P_g @ h2[g]
    S[g,:] = h3[g][g,:] @ h3[g].T      (row g of supplement, pre-tril)
    out    = x + 0.5*(tril(S) + tril(S).T)

8 cores x 48 slots, raw Bass (manual semaphores), all-bf16 matmuls.
Because the host applies tril, slot s only needs S[g, i] for i <= g.  Slots
are mapped cyclically: slot s on core c handles context g = c + 8*(47-s), so
a single SPMD program can use per-SLOT (core-independent) free sizes
E_s = 8*(47-s) + 10 >= g+3 for mm3/mm4 while staying load-balanced.  Per
slot (software-pipelined, skew 3):
    mm1  A_T[h,:]  = sum_j H0[j,h] Pt[j,:]  (+ rank-1 W1r (x) pcol)  N=384
    mm2  h2[j,k]   = sum_h A_T[h,j] W2[h,k]                          N=128
    mm3  h3T[k,:]  = sum_j h2[j,k] PtAug[j,0:E]                      N=E_s
    mm4  S[1,:]    = sum_k d[k] h3T[k,:]   (psum row aliased)        N=E_s
PtAug chunk layout (W=388): cols 0,1 = P_g[g, j] (so h3T col 0 = d =
h3[g][g,:], core-independent), cols 2..385 = Pt, 386/387 pad -- keeps every
matmul slice 4-byte aligned.  tril/symmetrize/(+x) happen on host.
Startup: ~10 garbage warm-up matmuls keep the PE HAM busy (so real work
starts at 2.4GHz, not the cold 1.2GHz), and the initial h0f/pt0 loads are
split across 4 DMA queues.  PE stream at iter i: mm1(i), mm2(i-1),
mm3(i-2), mm4(i-3).
"""

import os
from contextlib import ExitStack

import numpy as np
import ml_dtypes

import concourse.bass as bass
import concourse.mybir as mybir
from concourse.bass_utils import run_bass_kernel_spmd

N = 384
D = 128
HID = 256
HID2 = 128
NCORES = 8
NB = N // NCORES  # 48 slots per core
W = N + 4  # pt chunk width: 2 dup prow cols + 384 Pt cols + 2 pad
PTBUF = 8  # pt SBUF ring depth
SRBUF = 8  # S-row SBUF ring depth
NWARM = 10  # garbage matmuls to pre-warm the PE HAM clock gate
PCA = 8  # slots whose pcol rows load in the first pcall DMA

# per-slot mm3/mm4 free size: 2 prow cols + (g_max+1) needed cols, g_max =
# 8*(47-s)+7 -> E_s = 8*(47-s)+10  (even, <= 386)
ES = [8 * (NB - 1 - s) + 10 for s in range(NB)]

F32 = mybir.dt.float32
BF16 = mybir.dt.bfloat16
AFT = mybir.ActivationFunctionType

_NC_CACHE = {}
LAST_RESULT = None  # test.py reads exec_time_ns from here


def _pt_thr(slot: int) -> int:
    """sem_pt[slot % PTBUF] value after the fill for `slot` completes.

    Slot 0 is loaded by three chunk DMAs (3 x 16); all others by one.
    """
    n_fills = slot // PTBUF + 1
    return 16 * n_fills + (32 if slot % PTBUF == 0 else 0)


def _build_nc() -> bass.Bass:
    nc = bass.Bass()
    pt_d = nc.declare_dram_parameter("pt", [NB, 128, 3 * W], BF16, isOutput=False)
    pcol_d = nc.declare_dram_parameter("pcol", [1, NB * N], BF16, isOutput=False)
    h0f_d = nc.declare_dram_parameter("h0f", [128, 3 * HID], BF16, isOutput=False)
    w1r_d = nc.declare_dram_parameter("w1r", [1, HID], BF16, isOutput=False)
    w2f_d = nc.declare_dram_parameter("w2f", [128, 2 * HID2], BF16, isOutput=False)
    out_ds = [
        nc.declare_dram_parameter(f"o{b:02d}", [1, ES[b]], F32, isOutput=True)
        for b in range(NB)
    ]

    ctx = ExitStack()
    with ctx:
        # ---- persistent SBUF ----
        h0f = ctx.enter_context(nc.sbuf_tensor("h0f_s", [128, 3 * HID], BF16))
        w1r = ctx.enter_context(nc.sbuf_tensor("w1r_s", [1, HID], BF16))
        w2f = ctx.enter_context(nc.sbuf_tensor("w2f_s", [128, 2 * HID2], BF16))
        pcall = ctx.enter_context(nc.sbuf_tensor("pcall_s", [1, NB * N], BF16))
        pt = [
            ctx.enter_context(nc.sbuf_tensor(f"ptb{s}", [128, 3 * W], BF16))
            for s in range(PTBUF)
        ]
        at = [
            ctx.enter_context(nc.sbuf_tensor(f"atb{s}", [128, 2 * N], BF16))
            for s in range(3)
        ]
        h2sb = [
            ctx.enter_context(nc.sbuf_tensor(f"h2b{s}", [128, N], BF16))
            for s in range(3)
        ]
        h3sb = [
            ctx.enter_context(nc.sbuf_tensor(f"h3b{s}", [128, W], BF16))
            for s in range(3)
        ]
        srow = [
            ctx.enter_context(nc.sbuf_tensor(f"srowb{s}", [1, W], F32))
            for s in range(SRBUF)
        ]
        # ---- PSUM: 8 banks exactly ----
        aps = [
            [
                ctx.enter_context(
                    nc.psum_tensor(f"apsb{p}{h}", [128, N], F32)
                )
                for h in range(2)
            ]
            for p in range(2)
        ]  # aps[pair][hc]
        h2ps = [
            ctx.enter_context(nc.psum_tensor(f"h2psb{s}", [128, N], F32))
            for s in range(2)
        ]
        h3ps = [
            ctx.enter_context(nc.psum_tensor(f"h3psb{s}", [128, N + 2], F32))
            for s in range(2)
        ]

        # ---- semaphores ----
        sem_h0f = ctx.enter_context(nc.semaphore("sem_h0f"))
        sem_aux = ctx.enter_context(nc.semaphore("sem_aux"))
        sem_pc2 = ctx.enter_context(nc.semaphore("sem_pc2"))
        sem_w2 = ctx.enter_context(nc.semaphore("sem_w2"))
        sem_pt = [
            ctx.enter_context(nc.semaphore(f"sem_pt{s}")) for s in range(PTBUF)
        ]
        sem_out = [
            ctx.enter_context(nc.semaphore(f"sem_out{s}")) for s in range(SRBUF)
        ]
        sem_mm1 = ctx.enter_context(nc.semaphore("sem_mm1"))
        sem_relu = ctx.enter_context(nc.semaphore("sem_relu"))
        sem_mm2 = ctx.enter_context(nc.semaphore("sem_mm2"))
        sem_h2c = ctx.enter_context(nc.semaphore("sem_h2c"))
        sem_mm3 = ctx.enter_context(nc.semaphore("sem_mm3"))
        sem_h3c = ctx.enter_context(nc.semaphore("sem_h3c"))
        sem_mm4 = ctx.enter_context(nc.semaphore("sem_mm4"))
        sem_sc = ctx.enter_context(nc.semaphore("sem_sc"))

        block = ctx.enter_context(nc.Block())

        NI = NB + 3  # pipeline iterations (skew 3)

        @block.sync
        def _(sync):
            sync.dma_start(h0f[:, :], h0f_d[:, :]).then_inc(sem_h0f, 16)
            sync.dma_start(w1r[:, :], w1r_d[:, :]).then_inc(sem_aux, 16)
            for i in range(NI):
                k = i - 3
                if 0 <= k < NB:
                    sync.wait_ge(sem_sc, k + 1)
                    sync.dma_start(
                        out_ds[k][:, :], srow[k % SRBUF][:, 0 : ES[k]]
                    ).then_inc(sem_out[k % SRBUF], 16)

        @block.scalar
        def _(sc):
            sc.dma_start(pt[0][:, W : 2 * W], pt_d[0][:, W : 2 * W]).then_inc(
                sem_pt[0], 16
            )
            sc.dma_start(
                pcall[:, 0 : PCA * N], pcol_d[:, 0 : PCA * N]
            ).then_inc(sem_aux, 16)
            sc.dma_start(w2f[:, 0:HID2], w2f_d[:, 0:HID2]).then_inc(sem_w2, 16)
            sc.dma_start(w2f[:, HID2:], w2f_d[:, HID2:]).then_inc(sem_w2, 16)
            sc.dma_start(
                pcall[:, PCA * N :], pcol_d[:, PCA * N :]
            ).then_inc(sem_pc2, 16)
            for i in range(NI):
                k = i
                if k < NB:
                    if k >= 3:
                        sc.wait_ge(sem_mm2, k - 2)  # at[k%3] reuse
                    for hc in range(2):
                        sc.wait_ge(sem_mm1, 2 * k + hc + 1)
                        nc.scalar.activation(
                            at[k % 3][:, hc * N : (hc + 1) * N],
                            aps[k % 2][hc][:, :],
                            AFT.Relu,
                        ).then_inc(sem_relu, 1)

        @block.gpsimd
        def _(g):
            g.dma_start(pt[0][:, 0:W], pt_d[0][:, 0:W]).then_inc(sem_pt[0], 16)
            g.dma_start(pt[0][:, 2 * W :], pt_d[0][:, 2 * W :]).then_inc(
                sem_pt[0], 16
            )
            for p in range(1, min(PTBUF, NB)):
                if p >= 2:
                    # keep only ~2 prefetch DMAs in flight so early pt fills
                    # are not bandwidth-shared (rings interleave packets)
                    g.wait_ge(sem_pt[(p - 2) % PTBUF], _pt_thr(p - 2))
                g.dma_start(pt[p][:, :], pt_d[p]).then_inc(sem_pt[p], 16)
            for i in range(NI):
                p = i + PTBUF
                if p < NB:
                    g.wait_ge(sem_mm3, i + 1)
                    g.dma_start(
                        pt[p % PTBUF][:, :], pt_d[p]
                    ).then_inc(sem_pt[p % PTBUF], 16)

        @block.tensor
        def _(te):
            # ---- HAM warm-up: garbage matmuls while the startup DMAs run.
            # Data is whatever is in SBUF; aps[0][0] is overwritten by the
            # first real mm1 (start=True) before anything reads it. ----
            for _ in range(NWARM):
                nc.tensor.matmul(
                    aps[0][0][:, :],
                    h0f[:, 0:128],
                    pt[0][:, 2 : 2 + N],
                    start=True,
                    stop=True,
                    skip_group_check=True,
                )
            te.wait_ge(sem_h0f, 16)
            for i in range(NI):
                if i == 0:
                    te.wait_ge(sem_aux, 32)  # w1r + first pcall rows
                if i == 1:
                    te.wait_ge(sem_w2, 32)
                if i == PCA:
                    te.wait_ge(sem_pc2, 16)
                # ---- mm1(i): A_T chunks, bf16 N=384 ----
                if i < NB:
                    # aps-pair-reuse wait (sem_relu >= 2i-2) is implied by the
                    # previous iteration's wait before mm2.
                    te.wait_ge(sem_pt[i % PTBUF], _pt_thr(i))
                    ptt = pt[i % PTBUF]
                    for hc in range(2):
                        for t in range(3):
                            nc.tensor.matmul(
                                aps[i % 2][hc][:, :],
                                h0f[:, t * HID + hc * 128 : t * HID + hc * 128 + 128],
                                ptt[:, t * W + 2 : t * W + 2 + N],
                                start=(t == 0),
                                stop=False,
                                skip_group_check=True,
                            )

                # ---- mm2(i-1): h2 = A@W2, bf16 N=128 ----
                k = i - 1
                if 0 <= k < NB:
                    te.wait_ge(sem_relu, 2 * k + 2)
                    # h2ps[k%2]-reuse wait is implied by the previous
                    # iteration's wait before mm3.
                    dst = h2ps[k % 2]
                    for jc in range(3):
                        for ht in range(2):
                            mm = nc.tensor.matmul(
                                dst[:, jc * 128 : (jc + 1) * 128],
                                at[k % 3][
                                    :, ht * N + jc * 128 : ht * N + jc * 128 + 128
                                ],
                                w2f[:, ht * HID2 : (ht + 1) * HID2],
                                start=(ht == 0),
                                stop=(ht == 1),
                            )
                    if not (0 <= i - 2 < NB):
                        mm.then_inc(sem_mm2, 1)  # no mm3 rider this iter
                # ---- rank-1 pair for mm1(i), emitted after the short bf16
                # matmuls: the K=1 weight loads can't prefetch past a
                # full-height in-flight matmul, so placing them here turns two
                # ~250ns stalls into one small one ----
                if i < NB:
                    for hc in range(2):
                        r1 = nc.tensor.matmul(
                            aps[i % 2][hc][:, :],
                            w1r[:, hc * 128 : (hc + 1) * 128],
                            pcall[:, i * N : (i + 1) * N],
                            start=False,
                            stop=True,
                            skip_group_check=True,
                        )
                    r1.then_inc(sem_mm1, 2)
                # ---- mm3(i-2): h3T cols 0..E (col 0 = d), N=E_s ----
                k = i - 2
                if 0 <= k < NB:
                    E = ES[k]
                    te.wait_ge(sem_h2c, k + 1)
                    if k >= 2:
                        # h3ps[k%2]-reuse is implied by last iter's mm4 wait
                        te.wait_ge(sem_sc, k - 1)  # aliased S row was drained
                    dst = h3ps[k % 2]
                    ptt = pt[k % PTBUF]
                    for t in range(3):
                        mm = nc.tensor.matmul(
                            dst[:, 0:E],
                            h2sb[k % 3][:, t * 128 : (t + 1) * 128],
                            ptt[:, t * W : t * W + E],
                            start=(t == 0),
                            stop=(t == 2),
                        )
                        if t == 0 and k + 1 < NB:
                            # completion implies same-iter mm2(k+1) drained
                            mm.then_inc(sem_mm2, 1)
                    mm.then_inc(sem_mm3, 1)
                # ---- mm4(i-3): S row into h3ps[k%2] partition 0 ----
                k = i - 3
                if 0 <= k < NB:
                    E = ES[k]
                    te.wait_ge(sem_h3c, k + 1)
                    mm = nc.tensor.matmul(
                        h3ps[k % 2][0:1, 0:E],
                        h3sb[k % 3][:, 0:1],
                        h3sb[k % 3][:, 0:E],
                        start=True,
                        stop=True,
                    )
                    mm.then_inc(sem_mm4, 1)

        @block.vector
        def _(ve):
            for i in range(NI):
                k = i - 1
                if 0 <= k < NB:
                    if k >= 3:
                        ve.wait_ge(sem_mm3, k - 2)  # h2sb[k%3] reuse
                    ve.wait_ge(sem_mm2, k + 1)
                    nc.vector.tensor_copy(
                        h2sb[k % 3][:, :], h2ps[k % 2][:, :]
                    ).then_inc(sem_h2c, 1)
                k = i - 2
                if 0 <= k < NB:
                    if k >= 3:
                        ve.wait_ge(sem_mm4, k - 2)  # h3sb[k%3] reuse
                    ve.wait_ge(sem_mm3, k + 1)
                    nc.vector.tensor_copy(
                        h3sb[k % 3][:, 0 : ES[k]], h3ps[k % 2][:, 0 : ES[k]]
                    ).then_inc(sem_h3c, 1)
                k = i - 3
                if 0 <= k < NB:
                    ve.wait_ge(sem_mm4, k + 1)
                    if k >= SRBUF:
                        ve.wait_ge(sem_out[k % SRBUF], 16 * (k // SRBUF))
                    nc.vector.tensor_copy(
                        srow[k % SRBUF][:, 0 : ES[k]], h3ps[k % 2][0:1, 0 : ES[k]]
                    ).then_inc(sem_sc, 1)

    return nc


def _get_nc() -> bass.Bass:
    if "nc" not in _NC_CACHE:
        _NC_CACHE["nc"] = _build_nc()
    return _NC_CACHE["nc"]


def kernel(z, x, partials, W1, W2):
    global LAST_RESULT
    z = np.asarray(z, dtype=np.float32)
    x = np.asarray(x, dtype=np.float32)
    partials = np.asarray(partials, dtype=np.float32)
    W1 = np.asarray(W1, dtype=np.float32)
    W2 = np.asarray(W2, dtype=np.float32)

    H0 = z[0] @ W1[:D]  # [384, 256]
    h0f = (
        np.ascontiguousarray(H0.reshape(3, 128, HID).transpose(1, 0, 2))
        .reshape(128, 3 * HID)
        .astype(ml_dtypes.bfloat16)
    )
    w1r = np.ascontiguousarray(W1[D : D + 1]).astype(ml_dtypes.bfloat16)
    w2f = (
        np.ascontiguousarray(W2.reshape(2, 128, HID2).transpose(1, 0, 2))
        .reshape(128, 2 * HID2)
        .astype(ml_dtypes.bfloat16)
    )

    ptT = np.ascontiguousarray(partials.transpose(0, 2, 1))  # ptT[g,j,i]=P_g[i,j]
    ar = np.arange(N)
    prow = partials[ar, ar, :]  # [384, 384]  P_g[g, :]  (as fn of j)
    pcol = ptT[ar, ar, :]  # [384, 384]  P_g[:, g]  (as fn of i)

    in_maps = []
    for c in range(NCORES):
        # slot s on core c handles context g = c + 8*(47-s)
        gs = np.array([c + NCORES * (NB - 1 - s) for s in range(NB)])
        aug = np.zeros((NB, 3, 128, W), dtype=ml_dtypes.bfloat16)
        aug[..., 2 : 2 + N] = ptT[gs].reshape(NB, 3, 128, N).astype(
            ml_dtypes.bfloat16
        )
        pr = prow[gs].reshape(NB, 3, 128).astype(ml_dtypes.bfloat16)
        aug[..., 0] = pr
        aug[..., 1] = pr
        aug = np.ascontiguousarray(aug.transpose(0, 2, 1, 3)).reshape(
            NB, 128, 3 * W
        )
        in_maps.append(
            {
                "pt": aug,
                "pcol": np.ascontiguousarray(pcol[gs])
                .astype(ml_dtypes.bfloat16)
                .reshape(1, NB * N),
                "h0f": h0f,
                "w1r": w1r,
                "w2f": w2f,
            }
        )

    nc = _get_nc()
    res = run_bass_kernel_spmd(
        nc,
        in_maps,
        core_ids=list(range(NCORES)),
        trace=bool(os.environ.get("KERNEL_TRACE")),
    )
    LAST_RESULT = res
    S = np.zeros((N, N), dtype=np.float32)
    for c in range(NCORES):
        for s in range(NB):
            g = c + NCORES * (NB - 1 - s)
            row = np.asarray(res.results[c][f"o{s:02d}"], np.float32)[0]
            S[g, 0 : g + 1] = row[2 : 3 + g]
    sup = S + S.T  # S is already lower-triangular (diag doubled, as in TF)
    sup = sup * np.float32(0.5)
    return (x + sup).astype(np.float32)
